# revision 2
# baseline (speedup 1.0000x reference)
"""LocalAttention (N=8192, D=512) on 8 Trainium2 NeuronCores.

Strategy (sequence-parallel over query rows, per the sharding hint):
  - each core owns 1024 query rows; K^T/V are computed replicated on-chip
  - all matmuls run in fp32r (TF32-like: 11-bit mantissa, 1 cyc/row at N=512)
  - softmax in S-layout: rowsum via ACT-exp accum_out (free), mask applied as
    a -60*sqrt(D) additive bias (bf16) on raw scores before exp
  - attn = EM * (1/rowsum) via tensor_scalar (per-partition scalar), exact fp32
  - AV needs EM^T: PE transposes -> f32r evac -> HBM round-trip (SBUF cannot
    hold K^T and V simultaneously); phase B streams EM^T back against resident V
Outputs: (cntx [8192,512] f32, attn [8192,8192] f32), matching reference().
"""
import sys
sys.path.insert(0, '/opt/trn_rl_repo')

import numpy as np
import ml_dtypes

import concourse.bacc as bacc
import concourse.tile as tile
from concourse import mybir
from concourse.bass_utils import run_bass_kernel_spmd

F32 = mybir.dt.float32
F32R = mybir.dt.float32r
BF16 = mybir.dt.bfloat16
AF = mybir.ActivationFunctionType

N = 8192
D = 512
NCORES = 8
MS = N // NCORES           # 1024 rows per core
RT = MS // 128             # 8 row tiles per core
NB = N // 512              # 16 n-slices of 512
SCALE = 1.0 / np.sqrt(np.float32(D))   # 0.0441941...
MASKVAL = -60.0 * float(np.sqrt(np.float64(D)))  # raw-score bias => -60 after scale


def _round_f32r(a: np.ndarray) -> np.ndarray:
    u = np.ascontiguousarray(a, dtype=np.float32).view(np.uint32)
    r = (u + 0x7FF + ((u >> 12) & 1)) & np.uint32(0xFFFFF000)
    return r.view(np.float32)


_CACHE = {}


def _build():
    if "nc" in _CACHE:
        return _CACHE["nc"]
    nc = bacc.Bacc("TRN2", target_bir_lowering=False, debug=False,
                   num_devices=NCORES)
    xt_d = nc.dram_tensor("xt", [D, N], F32R, kind="ExternalInput").ap()
    xts_d = nc.dram_tensor("xts", [D, MS], F32R, kind="ExternalInput").ap()
    wq_d = nc.dram_tensor("wqT", [D, D], F32R, kind="ExternalInput").ap()
    wk_d = nc.dram_tensor("wkT", [D, D], F32R, kind="ExternalInput").ap()
    wv_d = nc.dram_tensor("wvT", [D, D], F32R, kind="ExternalInput").ap()
    mb_d = nc.dram_tensor("maskb", [MS, N], BF16, kind="ExternalInput").ap()
    id_d = nc.dram_tensor("idn", [128, 128], F32, kind="ExternalInput").ap()
    at_d = nc.dram_tensor("attn", [MS, N], F32, kind="ExternalOutput").ap()
    cx_d = nc.dram_tensor("cntx", [MS, D], F32, kind="ExternalOutput").ap()
    emt_d = nc.dram_tensor("emt_scratch", [N, MS], F32R)
    v_d = nc.dram_tensor("v_scratch", [N, D], F32R)

    with tile.TileContext(nc) as tc:
        with tc.tile_pool(name="persist", bufs=1) as persist:
            id_t = persist.tile([128, 128], F32)
            nc.sync.dma_start(id_t[:], id_d)
            rinv_all = persist.tile([128, RT], F32)

            with tc.tile_pool(name="ktp", bufs=1) as ktp, \
                 tc.tile_pool(name="qtp", bufs=1) as qtp:
                kt_t = ktp.tile([128, 4 * N], F32R)      # K^T: 4 do-chunks x N
                qt_t = qtp.tile([128, 4 * MS], F32R)     # Q^T: 4 do-chunks x MS

                # ---------- prologue: projections (replicated K^T, V; own Q^T)
                with tc.tile_pool(name="wts", bufs=1) as wts, \
                     tc.tile_pool(name="xtp", bufs=2) as xtp, \
                     tc.tile_pool(name="stg", bufs=3) as stg, \
                     tc.tile_pool(name="psp", bufs=4, space="PSUM") as psp:
                    wk_t = wts.tile([128, 4 * D], F32R)
                    wv_t = wts.tile([128, 4 * D], F32R)
                    wq_t = wts.tile([128, 4 * D], F32R)
                    for w_t, w_d in ((wk_t, wk_d), (wv_t, wv_d), (wq_t, wq_d)):
                        nc.sync.dma_start(
                            w_t[:].rearrange("p (c n) -> p c n", c=4),
                            w_d.rearrange("(c p) n -> p c n", p=128))

                    eng = [nc.vector, nc.scalar]
                    ei = 0
                    for nb in range(NB):
                        xt_t = xtp.tile([128, 4 * 512], F32R)
                        nc.sync.dma_start(
                            xt_t[:].rearrange("p (c n) -> p c n", c=4),
                            xt_d[:, nb * 512:(nb + 1) * 512].rearrange(
                                "(c p) n -> p c n", p=128))
                        # K^T[:, nb*512:+512]: 4 do-tiles
                        for d4 in range(4):
                            p = psp.tile([128, 512], F32, tag="pp")
                            for ki in range(4):
                                nc.tensor.matmul(
                                    p[:],
                                    wk_t[:, ki * D + d4 * 128: ki * D + (d4 + 1) * 128],
                                    xt_t[:, ki * 512:(ki + 1) * 512],
                                    start=(ki == 0), stop=(ki == 3))
                            if ei % 2 == 0:
                                nc.vector.tensor_copy(
                                    kt_t[:, d4 * N + nb * 512: d4 * N + (nb + 1) * 512], p[:])
                            else:
                                nc.scalar.copy(
                                    kt_t[:, d4 * N + nb * 512: d4 * N + (nb + 1) * 512], p[:])
                            ei += 1
                        # V rows nb*512..: 4 n-subchunks of 128
                        for s in range(4):
                            p = psp.tile([128, 512], F32, tag="pp")
                            for ki in range(4):
                                nc.tensor.matmul(
                                    p[:],
                                    xt_t[:, ki * 512 + s * 128: ki * 512 + (s + 1) * 128],
                                    wv_t[:, ki * D:(ki + 1) * D],
                                    start=(ki == 0), stop=(ki == 3))
                            v_t = stg.tile([128, 512], F32R, tag="vs")
                            if ei % 2 == 0:
                                nc.vector.tensor_copy(v_t[:], p[:])
                            else:
                                nc.scalar.copy(v_t[:], p[:])
                            ei += 1
                            nc.sync.dma_start(
                                v_d[nb * 512 + s * 128: nb * 512 + (s + 1) * 128, :], v_t[:])
                    # Q^T from own shard
                    for h in range(2):
                        xt_t = xtp.tile([128, 4 * 512], F32R)
                        nc.sync.dma_start(
                            xt_t[:].rearrange("p (c n) -> p c n", c=4),
                            xts_d[:, h * 512:(h + 1) * 512].rearrange(
                                "(c p) n -> p c n", p=128))
                        for d4 in range(4):
                            p = psp.tile([128, 512], F32, tag="pp")
                            for ki in range(4):
                                nc.tensor.matmul(
                                    p[:],
                                    wq_t[:, ki * D + d4 * 128: ki * D + (d4 + 1) * 128],
                                    xt_t[:, ki * 512:(ki + 1) * 512],
                                    start=(ki == 0), stop=(ki == 3))
                            if ei % 2 == 0:
                                nc.vector.tensor_copy(
                                    qt_t[:, d4 * MS + h * 512: d4 * MS + (h + 1) * 512], p[:])
                            else:
                                nc.scalar.copy(
                                    qt_t[:, d4 * MS + h * 512: d4 * MS + (h + 1) * 512], p[:])
                            ei += 1

                # ---------- phase A: scores, softmax, attn out, EM^T out
                with tc.tile_pool(name="emp", bufs=1) as emp, \
                     tc.tile_pool(name="mkp", bufs=2) as mkp, \
                     tc.tile_pool(name="smp", bufs=2) as smp, \
                     tc.tile_pool(name="etp", bufs=3) as etp, \
                     tc.tile_pool(name="psA", bufs=2, space="PSUM") as psA, \
                     tc.tile_pool(name="psT", bufs=2, space="PSUM") as psT:
                    em_t = emp.tile([128, N], F32)
                    rsparts = emp.tile([128, 8], F32)
                    rs_t = emp.tile([128, 1], F32)
                    ri_t = emp.tile([128, 1], F32)
                    for rt in range(RT):
                        for g in range(8):   # groups of 2 n-slices -> [128,1024]
                            p_s = psA.tile([128, 1024], F32, tag="ps_s")
                            for s in range(2):
                                nb = g * 2 + s
                                for ki in range(4):
                                    nc.tensor.matmul(
                                        p_s[:, s * 512:(s + 1) * 512],
                                        qt_t[:, ki * MS + rt * 128: ki * MS + (rt + 1) * 128],
                                        kt_t[:, ki * N + nb * 512: ki * N + (nb + 1) * 512],
                                        start=(ki == 0), stop=(ki == 3))
                            mb_t = mkp.tile([128, 1024], BF16)
                            nc.sync.dma_start(
                                mb_t[:],
                                mb_d[rt * 128:(rt + 1) * 128, g * 1024:(g + 1) * 1024])
                            sm_t = smp.tile([128, 1024], F32, tag="sm")
                            nc.vector.tensor_add(sm_t[:], p_s[:], mb_t[:])
                            nc.scalar.activation(
                                em_t[:, g * 1024:(g + 1) * 1024], sm_t[:],
                                AF.Exp, scale=float(SCALE),
                                accum_out=rsparts[:, g:g + 1])
                            for s in range(2):
                                nb = g * 2 + s
                                p_t = psT.tile([128, 512], F32, tag="ps_t")
                                for j in range(4):
                                    nc.tensor.transpose(
                                        p_t[:, j * 128:(j + 1) * 128],
                                        em_t[:, nb * 512 + j * 128: nb * 512 + (j + 1) * 128],
                                        id_t[:])
                                emt_t = etp.tile([128, 512], F32R, tag="et")
                                if s == 0:
                                    nc.vector.tensor_copy(emt_t[:], p_t[:])
                                else:
                                    nc.scalar.copy(emt_t[:], p_t[:])
                                nc.sync.dma_start(
                                    emt_d[nb * 512:(nb + 1) * 512,
                                          rt * 128:(rt + 1) * 128].rearrange(
                                        "(j p) m -> p j m", p=128),
                                    emt_t[:].rearrange("p (j m) -> p j m", j=4))
                        nc.vector.reduce_sum(rs_t[:], rsparts[:],
                                             axis=mybir.AxisListType.X)
                        nc.vector.reciprocal(ri_t[:], rs_t[:])
                        nc.vector.tensor_copy(rinv_all[:, rt:rt + 1], ri_t[:])
                        for g in range(8):
                            at_t = smp.tile([128, 1024], F32, tag="sm")
                            nc.vector.tensor_scalar_mul(
                                at_t[:], em_t[:, g * 1024:(g + 1) * 1024], ri_t[:])
                            nc.sync.dma_start(
                                at_d[rt * 128:(rt + 1) * 128,
                                     g * 1024:(g + 1) * 1024], at_t[:])

            # ---------- phase B: cntx = (EM @ V) * rinv
            with tc.tile_pool(name="vp", bufs=1) as vp, \
                 tc.tile_pool(name="eip", bufs=2) as eip, \
                 tc.tile_pool(name="cxp", bufs=2) as cxp, \
                 tc.tile_pool(name="psB", bufs=2, space="PSUM") as psB:
                v_t = vp.tile([128, 64 * 512], F32R)
                nc.sync.dma_start(
                    v_t[:].rearrange("p (j n) -> p j n", j=64),
                    v_d.rearrange("(j p) n -> p j n", p=128))
                for rt in range(RT):
                    ei_t = eip.tile([128, 64 * 128], F32R)
                    nc.sync.dma_start(
                        ei_t[:].rearrange("p (j m) -> p j m", j=64),
                        emt_d[:, rt * 128:(rt + 1) * 128].rearrange(
                            "(j p) m -> p j m", p=128))
                    p_c = psB.tile([128, 512], F32, tag="ps_c")
                    for j in range(64):
                        nc.tensor.matmul(
                            p_c[:],
                            ei_t[:, j * 128:(j + 1) * 128],
                            v_t[:, j * 512:(j + 1) * 512],
                            start=(j == 0), stop=(j == 63))
                    cx_t = cxp.tile([128, 512], F32)
                    nc.vector.tensor_scalar_mul(cx_t[:], p_c[:],
                                                rinv_all[:, rt:rt + 1])
                    nc.sync.dma_start(cx_d[rt * 128:(rt + 1) * 128, :], cx_t[:])

    nc.compile()
    _CACHE["nc"] = nc
    return nc


def kernel(x, mask, Wq, Wk, Wv):
    x = np.asarray(x, dtype=np.float32)
    mask = np.asarray(mask)
    nc = _build()

    xt = _round_f32r(np.ascontiguousarray(x.T))                 # [D, N]
    wqT = _round_f32r(np.ascontiguousarray(np.asarray(Wq, np.float32).T))
    wkT = _round_f32r(np.ascontiguousarray(np.asarray(Wk, np.float32).T))
    wvT = _round_f32r(np.ascontiguousarray(np.asarray(Wv, np.float32).T))
    idn = np.eye(128, dtype=np.float32)
    # bf16(-1357.645) == 0xC4AA; pure-integer select is ~2x faster than
    # np.where(f32).astype(bf16) on the 64M-element mask
    maskb = np.where(mask, np.uint16(0xC4AA), np.uint16(0)).view(
        ml_dtypes.bfloat16)                                     # [N, N]

    in_maps = []
    for c in range(NCORES):
        r0, r1 = c * MS, (c + 1) * MS
        in_maps.append({
            "xt": xt,
            "xts": np.ascontiguousarray(xt[:, r0:r1]),
            "wqT": wqT, "wkT": wkT, "wvT": wvT,
            "maskb": np.ascontiguousarray(maskb[r0:r1, :]),
            "idn": idn,
        })
    res = run_bass_kernel_spmd(nc, in_maps, list(range(NCORES)))
    attn = np.concatenate([res.results[c]["attn"] for c in range(NCORES)], axis=0)
    cntx = np.concatenate([res.results[c]["cntx"] for c in range(NCORES)], axis=0)
    return (cntx, attn)


# revision 3
# speedup vs baseline: 1.2523x; 1.2523x over previous
"""LocalAttention (N=8192, D=512) on 8 Trainium2 NeuronCores.

Strategy (sequence-parallel over query rows, per the sharding hint):
  - each core owns 1024 query rows; K^T/V are computed replicated on-chip
  - all matmuls run in fp32r (TF32-like: 11-bit mantissa, 1 cyc/row at N=512)
  - softmax in S-layout: rowsum via ACT-exp accum_out (free), mask applied as
    a -60*sqrt(D) additive bias (bf16) on raw scores before exp
  - attn = EM * (1/rowsum) via tensor_scalar (per-partition scalar), exact fp32
  - AV needs EM^T: PE transposes -> f32r evac -> HBM round-trip (SBUF cannot
    hold K^T and V simultaneously); phase B streams EM^T back against resident V
Outputs: (cntx [8192,512] f32, attn [8192,8192] f32), matching reference().
"""
import sys
sys.path.insert(0, '/opt/trn_rl_repo')

import numpy as np
import ml_dtypes

import concourse.bacc as bacc
import concourse.tile as tile
from concourse import mybir
from concourse.bass_utils import run_bass_kernel_spmd

F32 = mybir.dt.float32
F32R = mybir.dt.float32r
BF16 = mybir.dt.bfloat16
AF = mybir.ActivationFunctionType

N = 8192
D = 512
NCORES = 8
MS = N // NCORES           # 1024 rows per core
RT = MS // 128             # 8 row tiles per core
NB = N // 512              # 16 n-slices of 512
SCALE = 1.0 / np.sqrt(np.float32(D))   # 0.0441941...
MASKVAL = -60.0 * float(np.sqrt(np.float64(D)))  # raw-score bias => -60 after scale


def _round_f32r(a: np.ndarray) -> np.ndarray:
    u = np.ascontiguousarray(a, dtype=np.float32).view(np.uint32)
    r = (u + 0x7FF + ((u >> 12) & 1)) & np.uint32(0xFFFFF000)
    return r.view(np.float32)


_CACHE = {}


def _build():
    if "nc" in _CACHE:
        return _CACHE["nc"]
    nc = bacc.Bacc("TRN2", target_bir_lowering=False, debug=False,
                   num_devices=NCORES)
    xt_d = nc.dram_tensor("xt", [D, N], F32R, kind="ExternalInput").ap()
    xts_d = nc.dram_tensor("xts", [D, MS], F32R, kind="ExternalInput").ap()
    wq_d = nc.dram_tensor("wqT", [D, D], F32R, kind="ExternalInput").ap()
    wk_d = nc.dram_tensor("wkT", [D, D], F32R, kind="ExternalInput").ap()
    wv_d = nc.dram_tensor("wvT", [D, D], F32R, kind="ExternalInput").ap()
    mb_d = nc.dram_tensor("maskb", [MS, N], BF16, kind="ExternalInput").ap()
    id_d = nc.dram_tensor("idn", [128, 128], F32, kind="ExternalInput").ap()
    at_d = nc.dram_tensor("attn", [MS, N], F32, kind="ExternalOutput").ap()
    cx_d = nc.dram_tensor("cntx", [MS, D], F32, kind="ExternalOutput").ap()
    emt_d = nc.dram_tensor("emt_scratch", [N, MS], F32R)
    v_d = nc.dram_tensor("v_scratch", [N, D], F32R)

    with tile.TileContext(nc) as tc:
        with tc.tile_pool(name="persist", bufs=1) as persist:
            id_t = persist.tile([128, 128], F32)
            nc.sync.dma_start(id_t[:], id_d)
            rinv_all = persist.tile([128, RT], F32)

            with tc.tile_pool(name="ktp", bufs=1) as ktp, \
                 tc.tile_pool(name="qtp", bufs=1) as qtp:
                kt_t = ktp.tile([128, 4 * N], F32R)      # K^T: 4 do-chunks x N
                qt_t = qtp.tile([128, 4 * MS], F32R)     # Q^T: 4 do-chunks x MS

                # ---------- prologue: projections (replicated K^T, V; own Q^T)
                with tc.tile_pool(name="wts", bufs=1) as wts, \
                     tc.tile_pool(name="xtp", bufs=2) as xtp, \
                     tc.tile_pool(name="stg", bufs=3) as stg, \
                     tc.tile_pool(name="psp", bufs=4, space="PSUM") as psp:
                    wk_t = wts.tile([128, 4 * D], F32R)
                    wv_t = wts.tile([128, 4 * D], F32R)
                    wq_t = wts.tile([128, 4 * D], F32R)
                    for w_t, w_d in ((wk_t, wk_d), (wv_t, wv_d), (wq_t, wq_d)):
                        nc.sync.dma_start(
                            w_t[:].rearrange("p (c n) -> p c n", c=4),
                            w_d.rearrange("(c p) n -> p c n", p=128))

                    eng = [nc.vector, nc.scalar]
                    ei = 0
                    for nb in range(NB):
                        xt_t = xtp.tile([128, 4 * 512], F32R)
                        nc.sync.dma_start(
                            xt_t[:].rearrange("p (c n) -> p c n", c=4),
                            xt_d[:, nb * 512:(nb + 1) * 512].rearrange(
                                "(c p) n -> p c n", p=128))
                        # K^T[:, nb*512:+512]: 4 do-tiles
                        for d4 in range(4):
                            p = psp.tile([128, 512], F32, tag="pp")
                            for ki in range(4):
                                nc.tensor.matmul(
                                    p[:],
                                    wk_t[:, ki * D + d4 * 128: ki * D + (d4 + 1) * 128],
                                    xt_t[:, ki * 512:(ki + 1) * 512],
                                    start=(ki == 0), stop=(ki == 3))
                            if ei % 2 == 0:
                                nc.vector.tensor_copy(
                                    kt_t[:, d4 * N + nb * 512: d4 * N + (nb + 1) * 512], p[:])
                            else:
                                nc.scalar.copy(
                                    kt_t[:, d4 * N + nb * 512: d4 * N + (nb + 1) * 512], p[:])
                            ei += 1
                        # V rows nb*512..: 4 n-subchunks of 128
                        for s in range(4):
                            p = psp.tile([128, 512], F32, tag="pp")
                            for ki in range(4):
                                nc.tensor.matmul(
                                    p[:],
                                    xt_t[:, ki * 512 + s * 128: ki * 512 + (s + 1) * 128],
                                    wv_t[:, ki * D:(ki + 1) * D],
                                    start=(ki == 0), stop=(ki == 3))
                            v_t = stg.tile([128, 512], F32R, tag="vs")
                            if ei % 2 == 0:
                                nc.vector.tensor_copy(v_t[:], p[:])
                            else:
                                nc.scalar.copy(v_t[:], p[:])
                            ei += 1
                            nc.sync.dma_start(
                                v_d[nb * 512 + s * 128: nb * 512 + (s + 1) * 128, :], v_t[:])
                    # Q^T from own shard
                    for h in range(2):
                        xt_t = xtp.tile([128, 4 * 512], F32R)
                        nc.sync.dma_start(
                            xt_t[:].rearrange("p (c n) -> p c n", c=4),
                            xts_d[:, h * 512:(h + 1) * 512].rearrange(
                                "(c p) n -> p c n", p=128))
                        for d4 in range(4):
                            p = psp.tile([128, 512], F32, tag="pp")
                            for ki in range(4):
                                nc.tensor.matmul(
                                    p[:],
                                    wq_t[:, ki * D + d4 * 128: ki * D + (d4 + 1) * 128],
                                    xt_t[:, ki * 512:(ki + 1) * 512],
                                    start=(ki == 0), stop=(ki == 3))
                            if ei % 2 == 0:
                                nc.vector.tensor_copy(
                                    qt_t[:, d4 * MS + h * 512: d4 * MS + (h + 1) * 512], p[:])
                            else:
                                nc.scalar.copy(
                                    qt_t[:, d4 * MS + h * 512: d4 * MS + (h + 1) * 512], p[:])
                            ei += 1

                # ---------- phase A: scores, softmax, attn out, EM^T out
                with tc.tile_pool(name="emp", bufs=1) as emp, \
                     tc.tile_pool(name="mkp", bufs=2) as mkp, \
                     tc.tile_pool(name="smp", bufs=2) as smp, \
                     tc.tile_pool(name="etp", bufs=3) as etp, \
                     tc.tile_pool(name="psA", bufs=3, space="PSUM") as psA, \
                     tc.tile_pool(name="psT", bufs=2, space="PSUM") as psT:
                    em_t = emp.tile([128, N], F32)
                    rsparts = emp.tile([128, 8], F32)
                    rs_t = emp.tile([128, 1], F32)
                    ri_t = emp.tile([128, 1], F32)
                    for rt in range(RT):
                        for g in range(8):   # groups of 2 n-slices -> [128,1024]
                            p_s = psA.tile([128, 1024], F32, tag="ps_s")
                            for s in range(2):
                                nb = g * 2 + s
                                for ki in range(4):
                                    nc.tensor.matmul(
                                        p_s[:, s * 512:(s + 1) * 512],
                                        qt_t[:, ki * MS + rt * 128: ki * MS + (rt + 1) * 128],
                                        kt_t[:, ki * N + nb * 512: ki * N + (nb + 1) * 512],
                                        start=(ki == 0), stop=(ki == 3))
                            mb_t = mkp.tile([128, 1024], BF16)
                            nc.sync.dma_start(
                                mb_t[:],
                                mb_d[rt * 128:(rt + 1) * 128, g * 1024:(g + 1) * 1024])
                            sm_t = smp.tile([128, 1024], F32, tag="sm")
                            nc.vector.tensor_add(sm_t[:], p_s[:], mb_t[:])
                            nc.scalar.activation(
                                em_t[:, g * 1024:(g + 1) * 1024], sm_t[:],
                                AF.Exp, scale=float(SCALE),
                                accum_out=rsparts[:, g:g + 1])
                            for s in range(2):
                                nb = g * 2 + s
                                p_t = psT.tile([128, 512], F32, tag="ps_t")
                                for j in range(4):
                                    nc.tensor.transpose(
                                        p_t[:, j * 128:(j + 1) * 128],
                                        em_t[:, nb * 512 + j * 128: nb * 512 + (j + 1) * 128],
                                        id_t[:])
                                emt_t = etp.tile([128, 512], F32R, tag="et")
                                if s == 0:
                                    nc.vector.tensor_copy(emt_t[:], p_t[:])
                                else:
                                    nc.scalar.copy(emt_t[:], p_t[:])
                                nc.sync.dma_start(
                                    emt_d[nb * 512:(nb + 1) * 512,
                                          rt * 128:(rt + 1) * 128].rearrange(
                                        "(j p) m -> p j m", p=128),
                                    emt_t[:].rearrange("p (j m) -> p j m", j=4))
                        nc.vector.reduce_sum(rs_t[:], rsparts[:],
                                             axis=mybir.AxisListType.X)
                        nc.vector.reciprocal(ri_t[:], rs_t[:])
                        nc.vector.tensor_copy(rinv_all[:, rt:rt + 1], ri_t[:])
                        for g in range(8):
                            at_t = smp.tile([128, 1024], F32, tag="sm")
                            nc.vector.tensor_scalar_mul(
                                at_t[:], em_t[:, g * 1024:(g + 1) * 1024], ri_t[:])
                            nc.sync.dma_start(
                                at_d[rt * 128:(rt + 1) * 128,
                                     g * 1024:(g + 1) * 1024], at_t[:])

            # ---------- phase B: cntx = (EM @ V) * rinv
            with tc.tile_pool(name="vp", bufs=1) as vp, \
                 tc.tile_pool(name="eip", bufs=2) as eip, \
                 tc.tile_pool(name="cxp", bufs=2) as cxp, \
                 tc.tile_pool(name="psB", bufs=2, space="PSUM") as psB:
                v_t = vp.tile([128, 64 * 512], F32R)
                nc.sync.dma_start(
                    v_t[:].rearrange("p (j n) -> p j n", j=64),
                    v_d.rearrange("(j p) n -> p j n", p=128))
                for rt in range(RT):
                    ei_t = eip.tile([128, 64 * 128], F32R)
                    nc.sync.dma_start(
                        ei_t[:].rearrange("p (j m) -> p j m", j=64),
                        emt_d[:, rt * 128:(rt + 1) * 128].rearrange(
                            "(j p) m -> p j m", p=128))
                    p_c = psB.tile([128, 512], F32, tag="ps_c")
                    for j in range(64):
                        nc.tensor.matmul(
                            p_c[:],
                            ei_t[:, j * 128:(j + 1) * 128],
                            v_t[:, j * 512:(j + 1) * 512],
                            start=(j == 0), stop=(j == 63))
                    cx_t = cxp.tile([128, 512], F32)
                    nc.vector.tensor_scalar_mul(cx_t[:], p_c[:],
                                                rinv_all[:, rt:rt + 1])
                    nc.sync.dma_start(cx_d[rt * 128:(rt + 1) * 128, :], cx_t[:])

    nc.compile()
    _CACHE["nc"] = nc
    return nc


def kernel(x, mask, Wq, Wk, Wv):
    x = np.asarray(x, dtype=np.float32)
    mask = np.asarray(mask)
    nc = _build()

    xt = _round_f32r(np.ascontiguousarray(x.T))                 # [D, N]
    wqT = _round_f32r(np.ascontiguousarray(np.asarray(Wq, np.float32).T))
    wkT = _round_f32r(np.ascontiguousarray(np.asarray(Wk, np.float32).T))
    wvT = _round_f32r(np.ascontiguousarray(np.asarray(Wv, np.float32).T))
    idn = np.eye(128, dtype=np.float32)
    # bf16(-1357.645) == 0xC4AA; pure-integer select is ~2x faster than
    # np.where(f32).astype(bf16) on the 64M-element mask
    maskb = np.where(mask, np.uint16(0xC4AA), np.uint16(0)).view(
        ml_dtypes.bfloat16)                                     # [N, N]

    in_maps = []
    for c in range(NCORES):
        r0, r1 = c * MS, (c + 1) * MS
        in_maps.append({
            "xt": xt,
            "xts": np.ascontiguousarray(xt[:, r0:r1]),
            "wqT": wqT, "wkT": wkT, "wvT": wvT,
            "maskb": np.ascontiguousarray(maskb[r0:r1, :]),
            "idn": idn,
        })
    res = run_bass_kernel_spmd(nc, in_maps, list(range(NCORES)))
    attn = np.concatenate([res.results[c]["attn"] for c in range(NCORES)], axis=0)
    cntx = np.concatenate([res.results[c]["cntx"] for c in range(NCORES)], axis=0)
    return (cntx, attn)
